# revision 1
# baseline (speedup 1.0000x reference)
"""DreamAttention (GQA + RoPE + causal) on 8 trn2 NeuronCores.

Sharding: DP=2 over batch x sequence-parallel over q-tiles (no collectives).
Core c -> (batch b = c // 4, seq rank r = c % 4). Core r owns q-tiles
[r, 7-r, 8+r, 15-r] (128 rows each, ascending) — every core gets exactly 34
k-tile-blocks of causal attention work, so the load is perfectly balanced.
Each core computes ALL 28 heads for its 512 q rows and the FULL K/V
(redundantly, 4x) — that redundancy is far cheaper than a ReduceScatter of
o_proj partials over the (slow) inter-core links.

Per-core dataflow (all matmuls in fp32r = full-rate ~tf32 precision):
  - host supplies x^T tiles (D on partitions): xq (the core's q columns) and
    xt (full sequence, for K/V)
  - projections: QT [d, 28h, 512q], KT [d, 4kv, 2048], VT -> V via PE transpose
  - RoPE via SBUF->SBUF DMA partition rotation + DVE mul/add; host sends
    per-core-gathered cos/sin for q and full-sequence cos/sin for k
  - attention in transposed form per (head, k-tile): S^T[k, q-suffix] ->
    exp -> PV accumulates out^T[d, q]; the ascending q-tile layout makes the
    causally-live q columns a suffix, so each k-tile processes only [128, w]
    with w in {512, 384, 256, 128}; causality inside the suffix is enforced
    with a host-built additive mask; softmax sums via ones-matmul over a
    DVE-accumulated P^T running sum; normalization fused into the
    PSUM->SBUF move, which overwrites the spent Q slice
  - o_proj: attnT stationary, full Wo moving, accumulate over 28 head-chunks;
    output rows are core-owned -> DMA straight to the external output
Host reassembles the 8 cores' row-slices into the full [2, 2048, 3584] output.
"""

import math

import numpy as np

import concourse.bass as bass
import concourse.mybir as mybir
import concourse.tile as tile
from concourse import bacc
from concourse.bass_utils import run_bass_kernel_spmd
from concourse.masks import make_identity

F32 = mybir.dt.float32
F32R = mybir.dt.float32r

B, S, D = 2, 2048, 3584
H, KVH, HD = 28, 4, 128
ROPE_THETA = 1000000.0
GQ = H // KVH   # 7 q heads per kv head
DKT = D // 128  # 28 k-tiles over D
SC = 512        # s-chunk width for K/V projection
NSC = S // SC   # 4
NKT = S // 128  # 16 k tiles over sequence
NDC = 7         # output D chunks of 512
NQT = 4         # q-tiles owned per core
QW = NQT * 128  # 512 q columns per core
SCALE = 1.0 / math.sqrt(HD)


def _qtiles(r):
    """Ascending q-tile ids owned by seq-rank r; sum of (t+1) == 34 for all r."""
    return [r, 7 - r, 8 + r, 15 - r]


def _wof(kti):
    # Live-suffix width for k-tile kti. Rank-independent: every rank's
    # ascending tile list [t0<t1<t2<t3] satisfies t0<=3, 4<=t1<=7, 8<=t2<=11,
    # 12<=t3<=15, so #(tiles >= kti) == 4 - kti//4 for all ranks.
    return 128 * (4 - kti // 4)


_NC_CACHE = {}


def _build_nc(loop_n=1, phases="ABC"):
    key = ("nc", loop_n, phases)
    if key in _NC_CACHE:
        return _NC_CACHE[key]

    nc = bacc.Bacc("TRN2", target_bir_lowering=False, debug=False, num_devices=8)

    xq_d = nc.dram_tensor("xq", [DKT, 128, QW], F32R, kind="ExternalInput").ap()
    xt_d = nc.dram_tensor("xt", [NSC, DKT, 128, SC], F32R, kind="ExternalInput").ap()
    wq_d = nc.dram_tensor("wq", [H, 128, DKT, 128], F32R, kind="ExternalInput").ap()
    wkv_d = nc.dram_tensor(
        "wkv", [2 * KVH, 2, 128, DKT // 2, 128], F32R, kind="ExternalInput"
    ).ap()
    wo_d = nc.dram_tensor("wo", [NDC, DKT, 128, 512], F32R, kind="ExternalInput").ap()
    cosq_d = nc.dram_tensor("cosq", [128, QW], F32R, kind="ExternalInput").ap()
    sinq_d = nc.dram_tensor("sinq", [128, QW], F32R, kind="ExternalInput").ap()
    cosk_d = nc.dram_tensor("cosk", [128, S], F32R, kind="ExternalInput").ap()
    sink_d = nc.dram_tensor("sink", [128, S], F32R, kind="ExternalInput").ap()
    mask_d = nc.dram_tensor("mask", [NKT, 128, 128], F32, kind="ExternalInput").ap()
    out_d = nc.dram_tensor("out", [NQT, 128, D], F32, kind="ExternalOutput").ap()

    with tile.TileContext(nc) as tc:
        with (
            tc.tile_pool(name="persist", bufs=1) as persist,
            tc.tile_pool(name="ps_proj", bufs=2, space="PSUM") as ps_proj,
            tc.tile_pool(name="ps_s", bufs=3, space="PSUM") as ps_s,
            tc.tile_pool(name="ps_o", bufs=2, space="PSUM") as ps_o,
            tc.tile_pool(name="ps_sum", bufs=1, space="PSUM") as ps_sum,
        ):
            # qt doubles as the attention-output buffer: att(h) overwrites
            # qt[:, h, :] once head h's scores are done.
            qt = persist.tile([128, H, QW], F32R, name="qt")
            ident = persist.tile([128, 128], F32, name="ident")
            ones = persist.tile([128, 1], F32R, name="ones")
            ones_f = persist.tile([128, 1], F32, name="ones_f")

            make_identity(nc, ident)
            nc.vector.memset(ones_f, 1.0)
            nc.vector.tensor_copy(ones, ones_f)

            for _rep in range(loop_n):
                def rope(dst, cos_ap, sin_ap, width, tmp):
                    t = tmp[:, :width]
                    nc.gpsimd.dma_start(out=t[0:64, :], in_=dst[64:128, :])
                    nc.gpsimd.dma_start(out=t[64:128, :], in_=dst[0:64, :])
                    nc.vector.tensor_mul(t, t, sin_ap)
                    nc.vector.tensor_mul(dst, dst, cos_ap)
                    nc.vector.tensor_add(dst, dst, t)

                if "A" in phases:
                    # ---- Phase A1: Q projection + fused Q-RoPE ----
                    with (
                        tc.tile_pool(name="xqp", bufs=1) as xqp,
                        tc.tile_pool(name="wqp", bufs=3) as wqp,
                        tc.tile_pool(name="qtab", bufs=1) as qtab,
                        tc.tile_pool(name="qrtmp", bufs=3) as qrtmp,
                    ):
                        cosq = qtab.tile([128, QW], F32R, name="cosq")
                        sinq = qtab.tile([128, QW], F32R, name="sinq")
                        nc.scalar.dma_start(out=cosq, in_=cosq_d)
                        nc.scalar.dma_start(out=sinq, in_=sinq_d)
                        xq = xqp.tile([128, DKT, QW], F32R, name="xq")
                        nc.scalar.dma_start(
                            out=xq, in_=xq_d.rearrange("k p q -> p k q")
                        )
                        for ct in range(H):
                            wblk = wqp.tile([128, DKT, 128], F32R, name="wq")
                            nc.scalar.dma_start(out=wblk, in_=wq_d[ct])
                            psum = ps_proj.tile([128, QW], F32, name="pp")
                            for kti in range(DKT):
                                nc.tensor.matmul(
                                    psum,
                                    wblk[:, kti, :],
                                    xq[:, kti, :],
                                    start=(kti == 0),
                                    stop=(kti == DKT - 1),
                                )
                            nc.vector.tensor_copy(qt[:, ct, :], psum)
                            tmp = qrtmp.tile([128, QW], F32R, name="qrtmp")
                            rope(qt[:, ct, :], cosq, sinq, QW, tmp)

                kvp_cm = tc.tile_pool(name="kvp", bufs=1)
                kvp = kvp_cm.__enter__()
                kt_t = kvp.tile([128, KVH, S], F32R, name="kt")
                vn = kvp.tile([128, KVH, NKT, 128], F32R, name="vn")

                if "A" in phases:
                    # ---- Phase A2: K/V projection over the full sequence ----
                    with (
                        tc.tile_pool(name="xtp", bufs=30) as xtp,
                        tc.tile_pool(name="wkvp", bufs=2) as wkvp,
                        tc.tile_pool(name="vtp", bufs=1) as vtp,
                    ):
                        for sc in range(NSC):
                            xts = []
                            for kti in range(DKT):
                                xtile = xtp.tile([128, SC], F32R, name="xt")
                                nc.sync.dma_start(out=xtile, in_=xt_d[sc, kti])
                                xts.append(xtile)
                            vtc = vtp.tile([128, KVH, SC], F32, name="vtc")
                            for ct in range(2 * KVH):  # 0-3: K heads, 4-7: V
                                psum = ps_proj.tile([128, SC], F32, name="pp")
                                for hf in range(2):
                                    wblk = wkvp.tile(
                                        [128, DKT // 2, 128], F32R, name="wkv"
                                    )
                                    if ct < KVH:
                                        nc.sync.dma_start(
                                            out=wblk, in_=wkv_d[ct, hf]
                                        )
                                    else:
                                        nc.scalar.dma_start(
                                            out=wblk, in_=wkv_d[ct, hf]
                                        )
                                    for kti in range(DKT // 2):
                                        gkt = hf * (DKT // 2) + kti
                                        nc.tensor.matmul(
                                            psum,
                                            wblk[:, kti, :],
                                            xts[gkt],
                                            start=(gkt == 0),
                                            stop=(gkt == DKT - 1),
                                        )
                                if ct < KVH:
                                    dest = kt_t[:, ct, sc * SC : (sc + 1) * SC]
                                else:
                                    dest = vtc[:, ct - KVH, :]
                                nc.vector.tensor_copy(dest, psum)
                            # V^T -> V natural, per chunk (4 s-tiles x 4 heads)
                            for kv in range(KVH):
                                for sti in range(SC // 128):
                                    st = sc * (SC // 128) + sti
                                    ptr = ps_o.tile([128, QW], F32, name="po")
                                    nc.tensor.transpose(
                                        ptr[:, 0:128],
                                        vtc[:, kv, sti * 128 : (sti + 1) * 128],
                                        ident,
                                    )
                                    nc.vector.tensor_copy(
                                        vn[:, kv, st, :], ptr[:, 0:128]
                                    )

                    # ---- Phase A3: K-RoPE ----
                    with (
                        tc.tile_pool(name="ropetab", bufs=1) as ropetab,
                        tc.tile_pool(name="ropep", bufs=3) as ropep,
                    ):
                        cosk = ropetab.tile([128, S], F32R, name="cosk")
                        sink = ropetab.tile([128, S], F32R, name="sink")
                        nc.scalar.dma_start(out=cosk, in_=cosk_d)
                        nc.scalar.dma_start(out=sink, in_=sink_d)
                        for kv in range(KVH):
                            for c in range(NSC):
                                sl = slice(c * SC, (c + 1) * SC)
                                tmp = ropep.tile([128, 512], F32R, name="ropetmp")
                                rope(
                                    kt_t[:, kv, sl],
                                    cosk[:, sl],
                                    sink[:, sl],
                                    SC,
                                    tmp,
                                )

                if "B" in phases:
                    # ---- Phase B: attention, 28 heads on the core's 512 q ----
                    with (
                        tc.tile_pool(name="ptp", bufs=3) as ptp,
                        tc.tile_pool(name="smallp", bufs=2) as smallp,
                        tc.tile_pool(name="maskp", bufs=1) as maskp,
                    ):
                        mask_t = maskp.tile([128, NKT, 128], F32, name="mask_t")
                        nc.scalar.dma_start(
                            out=mask_t, in_=mask_d.rearrange("k p q -> p k q")
                        )
                        for h in range(H):
                            kv = h // GQ
                            psum_o = ps_o.tile([128, QW], F32, name="po")
                            psum_r = ps_sum.tile([1, QW], F32, name="psr")
                            for kti in range(NKT):
                                w = _wof(kti)
                                lo = QW - w
                                psum_s = ps_s.tile([128, QW], F32, name="pss")
                                nc.tensor.matmul(
                                    psum_s[:, lo:],
                                    kt_t[:, kv, kti * 128 : (kti + 1) * 128],
                                    qt[:, h, lo:],
                                    start=True,
                                    stop=True,
                                )
                                # only the first live block can be diagonal;
                                # the host mask is triangular there (or zero
                                # when this k-tile is not one of the core's
                                # q-tiles)
                                nc.vector.tensor_add(
                                    psum_s[:, lo : lo + 128],
                                    psum_s[:, lo : lo + 128],
                                    mask_t[:, kti, :],
                                )
                                pt = ptp.tile([128, QW], F32R, name="pt")
                                nc.scalar.activation(
                                    pt[:, lo:],
                                    psum_s[:, lo:],
                                    mybir.ActivationFunctionType.Exp,
                                )
                                nc.tensor.matmul(
                                    psum_o[:, lo:],
                                    vn[:, kv, kti, :],
                                    pt[:, lo:],
                                    start=(kti == 0),
                                    stop=(kti == NKT - 1),
                                )
                                nc.tensor.matmul(
                                    psum_r[:, lo:],
                                    ones,
                                    pt[:, lo:],
                                    start=(kti == 0),
                                    stop=(kti == NKT - 1),
                                )
                            rec = smallp.tile([1, QW], F32, name="rec")
                            nc.vector.reciprocal(rec, psum_r)
                            bcast = smallp.tile([128, QW], F32, name="bcast")
                            nc.gpsimd.partition_broadcast(bcast, rec)
                            # fused normalize + PSUM->SBUF, overwriting head
                            # h's spent q columns
                            nc.vector.tensor_mul(qt[:, h, :], psum_o, bcast)

                kvp_cm.__exit__(None, None, None)

                if "C" in phases:
                    # ---- Phase C: o_proj (full Wo); rows are core-owned ----
                    with (
                        tc.tile_pool(name="wop", bufs=56) as wop,
                        tc.tile_pool(name="outp", bufs=3) as outp,
                    ):
                        for dc in range(NDC):
                            wo_tiles = []
                            for ct in range(DKT):
                                wt = wop.tile([128, 512], F32R, name="wo")
                                if ct % 2 == 0:
                                    nc.sync.dma_start(out=wt, in_=wo_d[dc, ct])
                                else:
                                    nc.scalar.dma_start(out=wt, in_=wo_d[dc, ct])
                                wo_tiles.append(wt)
                            for q in range(NQT):
                                psum = ps_proj.tile([128, 512], F32, name="pp")
                                for ct in range(DKT):
                                    nc.tensor.matmul(
                                        psum,
                                        qt[:, ct, q * 128 : (q + 1) * 128],
                                        wo_tiles[ct],
                                        start=(ct == 0),
                                        stop=(ct == DKT - 1),
                                    )
                                ob = outp.tile([128, 512], F32, name="ob")
                                nc.vector.tensor_copy(ob, psum)
                                nc.sync.dma_start(
                                    out=out_d[q, :, dc * 512 : (dc + 1) * 512],
                                    in_=ob,
                                )

    nc.finalize()
    _NC_CACHE[key] = nc
    return nc


def _host_inputs(hidden_states, Wq, Wk, Wv, Wo):
    hidden = np.asarray(hidden_states, dtype=np.float32)
    Wq = np.asarray(Wq, dtype=np.float32) * np.float32(SCALE)
    Wk = np.asarray(Wk, dtype=np.float32)
    Wv = np.asarray(Wv, dtype=np.float32)
    Wo = np.asarray(Wo, dtype=np.float32)

    inv_freq = 1.0 / ROPE_THETA ** (np.arange(0, HD, 2, dtype=np.float32) / HD)
    t = np.arange(S, dtype=np.float32)
    freqs = np.outer(t, inv_freq)  # [S, 64]
    cos_t = np.cos(freqs.T)  # [64, S]
    sin_t = np.sin(freqs.T)
    cosk = np.concatenate([cos_t, cos_t], axis=0).astype(np.float32)  # [128, S]
    sink = np.concatenate([-sin_t, sin_t], axis=0).astype(np.float32)

    # shared weight layouts (identical for every core)
    wq = np.ascontiguousarray(
        Wq.reshape(DKT, 128, H, 128).transpose(2, 1, 0, 3)
    )  # [h, p, kt, c]
    wk4 = Wk.reshape(DKT, 128, KVH, 128)
    wv4 = Wv.reshape(DKT, 128, KVH, 128)
    wkv = np.empty((2 * KVH, 2, 128, DKT // 2, 128), np.float32)
    for ct in range(KVH):
        for hf in range(2):
            ktsl = slice(hf * (DKT // 2), (hf + 1) * (DKT // 2))
            wkv[ct, hf] = wk4[ktsl, :, ct, :].transpose(1, 0, 2)
            wkv[KVH + ct, hf] = wv4[ktsl, :, ct, :].transpose(1, 0, 2)
    wo = np.ascontiguousarray(
        Wo.reshape(DKT, 128, NDC, 512).transpose(2, 0, 1, 3)
    )  # [dc, ct, p, d]

    in_maps = []
    for core in range(8):
        b, r = core // 4, core % 4
        tiles = _qtiles(r)
        qpos = np.concatenate(
            [np.arange(t0 * 128, (t0 + 1) * 128) for t0 in tiles]
        )  # [512] ascending global q positions
        xq = np.ascontiguousarray(
            hidden[b][qpos].reshape(QW, DKT, 128).transpose(1, 2, 0)
        )  # [kt, p, q]
        xt = np.ascontiguousarray(
            hidden[b].reshape(NSC, SC, DKT, 128).transpose(0, 2, 3, 1)
        )
        cosq = np.ascontiguousarray(cosk[:, qpos])
        sinq = np.ascontiguousarray(sink[:, qpos])
        # mask[kt]: [128, 128] additive mask for the FIRST live block of the
        # suffix (columns QW-w .. QW-w+128). Triangular when that block's
        # q-tile equals kt (the diagonal), all-zero otherwise.
        mask = np.zeros((NKT, 128, 128), np.float32)
        for kti in range(NKT):
            lo = QW - _wof(kti)
            kk = kti * 128 + np.arange(128)[:, None]
            qq = qpos[None, lo : lo + 128]
            mask[kti] = np.where(kk <= qq, 0.0, -30000.0)
        in_maps.append(
            {
                "xq": xq,
                "xt": xt,
                "wq": wq,
                "wkv": wkv,
                "wo": wo,
                "cosq": cosq,
                "sinq": sinq,
                "cosk": cosk,
                "sink": sink,
                "mask": mask,
            }
        )
    return in_maps


def kernel(hidden_states, Wq, Wk, Wv, Wo, trace=False):
    nc = _build_nc()
    in_maps = _host_inputs(hidden_states, Wq, Wk, Wv, Wo)
    res = run_bass_kernel_spmd(nc, in_maps, list(range(8)), trace=trace)
    out = np.empty((B, S, D), dtype=np.float32)
    for core in range(8):
        b, r = core // 4, core % 4
        o = res.results[core]["out"]  # [NQT, 128, D]
        for j, t0 in enumerate(_qtiles(r)):
            out[b, t0 * 128 : (t0 + 1) * 128, :] = o[j]
    if trace:
        kernel.last_exec_time_ns = res.exec_time_ns
    return out



# revision 5
# speedup vs baseline: 1.0858x; 1.0858x over previous
"""DreamAttention (GQA + RoPE + causal) on 8 trn2 NeuronCores.

Sharding: DP=2 over batch x sequence-parallel over q-tiles (no collectives).
Core c -> (batch b = c // 4, seq rank r = c % 4). Core r owns q-tiles
[r, 7-r, 8+r, 15-r] (128 rows each, ascending) — every core gets exactly 34
k-tile-blocks of causal attention work, so the load is perfectly balanced.
Each core computes ALL 28 heads for its 512 q rows and the FULL K/V
(redundantly, 4x) — that redundancy is far cheaper than a ReduceScatter of
o_proj partials over the (slow) inter-core links.

v2: all matmul operands in BF16 (fp32r streams at ~0.76 ns/col on HW; bf16
streams at ~0.42 ns/col — a ~1.8x matmul-rate win, and FWL halves LDWEIGHTS).
PSUM accumulation stays fp32. Softmax normalize chain reordered
(broadcast-then-reciprocal on [128, QW] instead of a 3.3us 1-partition
reciprocal) and ps_sum double-buffered to kill a ~2.2us/head PE stall.

Per-core dataflow:
  - host supplies x^T tiles (D on partitions) in bf16: xq (the core's q
    columns) and xt (full sequence, for K/V)
  - projections: QT [d, 28h, 512q], KT [d, 4kv, 2048], VT -> V via PE
    transpose
  - RoPE via SBUF->SBUF DMA partition rotation + DVE mul/add
  - attention in transposed form per (head, k-tile): S^T[k, q-suffix] ->
    exp -> PV accumulates out^T[d, q]; causally-live q columns are a suffix
    (widths 512/384/256/128); diagonal handled by a host-built additive mask
  - o_proj: attnT stationary, full Wo moving, accumulate over 28 head-chunks
Host reassembles the 8 cores' row-slices into the full [2, 2048, 3584] output.
"""

import math

import numpy as np
import ml_dtypes

import concourse.bass as bass
import concourse.mybir as mybir
import concourse.tile as tile
from concourse import bacc
from concourse.bass_utils import run_bass_kernel_spmd
from concourse.masks import make_identity

F32 = mybir.dt.float32
BF16 = mybir.dt.bfloat16
BF16_NP = ml_dtypes.bfloat16

B, S, D = 2, 2048, 3584
H, KVH, HD = 28, 4, 128
ROPE_THETA = 1000000.0
GQ = H // KVH   # 7 q heads per kv head
DKT = D // 128  # 28 k-tiles over D
SC = 512        # s-chunk width for K/V projection
NSC = S // SC   # 4
NKT = S // 128  # 16 k tiles over sequence
NDC = 7         # output D chunks of 512
NQT = 4         # q-tiles owned per core
QW = NQT * 128  # 512 q columns per core
SCALE = 1.0 / math.sqrt(HD)


def _qtiles(r):
    """Ascending q-tile ids owned by seq-rank r; sum of (t+1) == 34 for all r."""
    return [r, 7 - r, 8 + r, 15 - r]


def _wof(kti):
    # Live-suffix width for k-tile kti. Rank-independent: every rank's
    # ascending tile list [t0<t1<t2<t3] satisfies t0<=3, 4<=t1<=7, 8<=t2<=11,
    # 12<=t3<=15, so #(tiles >= kti) == 4 - kti//4 for all ranks.
    return 128 * (4 - kti // 4)


_NC_CACHE = {}


def _build_nc(loop_n=1, phases="ABC"):
    key = ("nc", loop_n, phases)
    if key in _NC_CACHE:
        return _NC_CACHE[key]

    nc = bacc.Bacc("TRN2", target_bir_lowering=False, debug=False, num_devices=8)

    xq_d = nc.dram_tensor("xq", [DKT, 128, QW], BF16, kind="ExternalInput").ap()
    xt_d = nc.dram_tensor("xt", [NSC, DKT, 128, SC], BF16, kind="ExternalInput").ap()
    wq_d = nc.dram_tensor("wq", [H, 128, DKT, 128], BF16, kind="ExternalInput").ap()
    wkv_d = nc.dram_tensor(
        "wkv", [2 * KVH, 2, 128, DKT // 2, 128], BF16, kind="ExternalInput"
    ).ap()
    wo_d = nc.dram_tensor("wo", [NDC, DKT, 128, 512], BF16, kind="ExternalInput").ap()
    cosq_d = nc.dram_tensor("cosq", [128, QW], BF16, kind="ExternalInput").ap()
    sinq_d = nc.dram_tensor("sinq", [128, QW], BF16, kind="ExternalInput").ap()
    cosk_d = nc.dram_tensor("cosk", [128, S], BF16, kind="ExternalInput").ap()
    sink_d = nc.dram_tensor("sink", [128, S], BF16, kind="ExternalInput").ap()
    mask_d = nc.dram_tensor("mask", [NKT, 128, 128], F32, kind="ExternalInput").ap()
    out_d = nc.dram_tensor("out", [NQT, 128, D], F32, kind="ExternalOutput").ap()

    with tile.TileContext(nc) as tc:
        with (
            tc.tile_pool(name="persist", bufs=1) as persist,
            tc.tile_pool(name="ps_proj", bufs=2, space="PSUM") as ps_proj,
            tc.tile_pool(name="ps_s", bufs=2, space="PSUM") as ps_s,
            tc.tile_pool(name="ps_o", bufs=2, space="PSUM") as ps_o,
            tc.tile_pool(name="ps_sum", bufs=2, space="PSUM") as ps_sum,
        ):
            # qt doubles as the attention-output buffer: att(h) overwrites
            # qt[:, h, :] once head h's scores are done.
            qt = persist.tile([128, H, QW], BF16, name="qt")
            ident = persist.tile([128, 128], BF16, name="ident")
            ones = persist.tile([128, 1], BF16, name="ones")

            make_identity(nc, ident)
            nc.vector.memset(ones, 1.0)

            for _rep in range(loop_n):
                def rope(dst, cos_ap, sin_ap, width, tmp):
                    t = tmp[:, :width]
                    nc.gpsimd.dma_start(out=t[0:64, :], in_=dst[64:128, :])
                    nc.gpsimd.dma_start(out=t[64:128, :], in_=dst[0:64, :])
                    nc.vector.tensor_mul(t, t, sin_ap)
                    nc.vector.tensor_mul(dst, dst, cos_ap)
                    nc.vector.tensor_add(dst, dst, t)

                if "A" in phases:
                    # ---- Phase A1: Q projection + fused Q-RoPE ----
                    with (
                        tc.tile_pool(name="xqp", bufs=1) as xqp,
                        tc.tile_pool(name="wqp", bufs=3) as wqp,
                        tc.tile_pool(name="qtab", bufs=1) as qtab,
                        tc.tile_pool(name="qrtmp", bufs=3) as qrtmp,
                    ):
                        cosq = qtab.tile([128, QW], BF16, name="cosq")
                        sinq = qtab.tile([128, QW], BF16, name="sinq")
                        nc.scalar.dma_start(out=cosq, in_=cosq_d)
                        nc.scalar.dma_start(out=sinq, in_=sinq_d)
                        xq = xqp.tile([128, DKT, QW], BF16, name="xq")
                        nc.scalar.dma_start(
                            out=xq, in_=xq_d.rearrange("k p q -> p k q")
                        )
                        for ct in range(H):
                            wblk = wqp.tile([128, DKT, 128], BF16, name="wq")
                            nc.scalar.dma_start(out=wblk, in_=wq_d[ct])
                            psum = ps_proj.tile([128, QW], F32, name="pp")
                            for kti in range(DKT):
                                nc.tensor.matmul(
                                    psum,
                                    wblk[:, kti, :],
                                    xq[:, kti, :],
                                    start=(kti == 0),
                                    stop=(kti == DKT - 1),
                                )
                            nc.vector.tensor_copy(qt[:, ct, :], psum)
                            tmp = qrtmp.tile([128, QW], BF16, name="qrtmp")
                            rope(qt[:, ct, :], cosq, sinq, QW, tmp)

                kvp_cm = tc.tile_pool(name="kvp", bufs=1)
                kvp = kvp_cm.__enter__()
                kt_t = kvp.tile([128, KVH, S], BF16, name="kt")
                vn = kvp.tile([128, KVH, NKT, 128], BF16, name="vn")

                if "A" in phases:
                    # ---- Phase A2: K/V projection over the full sequence ----
                    with (
                        tc.tile_pool(name="xtp", bufs=58) as xtp,
                        tc.tile_pool(name="wkvp", bufs=2) as wkvp,
                        tc.tile_pool(name="vtp", bufs=1) as vtp,
                    ):
                        for sc in range(NSC):
                            xts = []
                            for kti in range(DKT):
                                xtile = xtp.tile([128, SC], BF16, name="xt")
                                nc.sync.dma_start(out=xtile, in_=xt_d[sc, kti])
                                xts.append(xtile)
                            vtc = vtp.tile([128, KVH, SC], BF16, name="vtc")
                            for ct in range(2 * KVH):  # 0-3: K heads, 4-7: V
                                psum = ps_proj.tile([128, SC], F32, name="pp")
                                for hf in range(2):
                                    wblk = wkvp.tile(
                                        [128, DKT // 2, 128], BF16, name="wkv"
                                    )
                                    if ct < KVH:
                                        nc.sync.dma_start(
                                            out=wblk, in_=wkv_d[ct, hf]
                                        )
                                    else:
                                        nc.scalar.dma_start(
                                            out=wblk, in_=wkv_d[ct, hf]
                                        )
                                    for kti in range(DKT // 2):
                                        gkt = hf * (DKT // 2) + kti
                                        nc.tensor.matmul(
                                            psum,
                                            wblk[:, kti, :],
                                            xts[gkt],
                                            start=(gkt == 0),
                                            stop=(gkt == DKT - 1),
                                        )
                                if ct < KVH:
                                    dest = kt_t[:, ct, sc * SC : (sc + 1) * SC]
                                else:
                                    dest = vtc[:, ct - KVH, :]
                                nc.vector.tensor_copy(dest, psum)
                            # V^T -> V natural, per chunk (4 s-tiles x 4 heads)
                            for kv in range(KVH):
                                for sti in range(SC // 128):
                                    st = sc * (SC // 128) + sti
                                    ptr = ps_o.tile([128, 128], BF16, name="po")
                                    nc.tensor.transpose(
                                        ptr[:, 0:128],
                                        vtc[:, kv, sti * 128 : (sti + 1) * 128],
                                        ident,
                                    )
                                    nc.vector.tensor_copy(
                                        vn[:, kv, st, :], ptr[:, 0:128]
                                    )

                    # ---- Phase A3: K-RoPE ----
                    with (
                        tc.tile_pool(name="ropetab", bufs=1) as ropetab,
                        tc.tile_pool(name="ropep", bufs=3) as ropep,
                    ):
                        cosk = ropetab.tile([128, S], BF16, name="cosk")
                        sink = ropetab.tile([128, S], BF16, name="sink")
                        nc.scalar.dma_start(out=cosk, in_=cosk_d)
                        nc.scalar.dma_start(out=sink, in_=sink_d)
                        for kv in range(KVH):
                            for c in range(NSC):
                                sl = slice(c * SC, (c + 1) * SC)
                                tmp = ropep.tile([128, 512], BF16, name="ropetmp")
                                rope(
                                    kt_t[:, kv, sl],
                                    cosk[:, sl],
                                    sink[:, sl],
                                    SC,
                                    tmp,
                                )

                if "B" in phases:
                    # ---- Phase B: attention, 28 heads on the core's 512 q ----
                    with (
                        tc.tile_pool(name="ptp", bufs=3) as ptp,
                        tc.tile_pool(name="smallp", bufs=2) as smallp,
                        tc.tile_pool(name="maskp", bufs=1) as maskp,
                    ):
                        mask_t = maskp.tile([128, NKT, 128], F32, name="mask_t")
                        nc.scalar.dma_start(
                            out=mask_t, in_=mask_d.rearrange("k p q -> p k q")
                        )
                        for h in range(H):
                            kv = h // GQ
                            psum_o = ps_o.tile([128, QW], F32, name="po")
                            psum_r = ps_sum.tile([1, QW], F32, name="psr")
                            for kti in range(NKT):
                                w = _wof(kti)
                                lo = QW - w
                                psum_s = ps_s.tile([128, QW], F32, name="pss")
                                nc.tensor.matmul(
                                    psum_s[:, lo:],
                                    kt_t[:, kv, kti * 128 : (kti + 1) * 128],
                                    qt[:, h, lo:],
                                    start=True,
                                    stop=True,
                                )
                                # only the first live block can be diagonal;
                                # the host mask is triangular there (or zero
                                # when this k-tile is not one of the core's
                                # q-tiles)
                                nc.vector.tensor_add(
                                    psum_s[:, lo : lo + 128],
                                    psum_s[:, lo : lo + 128],
                                    mask_t[:, kti, :],
                                )
                                pt = ptp.tile([128, QW], BF16, name="pt")
                                nc.scalar.activation(
                                    pt[:, lo:],
                                    psum_s[:, lo:],
                                    mybir.ActivationFunctionType.Exp,
                                )
                                nc.tensor.matmul(
                                    psum_o[:, lo:],
                                    vn[:, kv, kti, :],
                                    pt[:, lo:],
                                    start=(kti == 0),
                                    stop=(kti == NKT - 1),
                                )
                                nc.tensor.matmul(
                                    psum_r[:, lo:],
                                    ones,
                                    pt[:, lo:],
                                    start=(kti == 0),
                                    stop=(kti == NKT - 1),
                                )
                            sum_sb = smallp.tile([1, QW], F32, name="sum_sb")
                            nc.scalar.copy(sum_sb, psum_r)
                            bcast = smallp.tile([128, QW], F32, name="bcast")
                            nc.gpsimd.partition_broadcast(bcast, sum_sb)
                            rec = smallp.tile([128, QW], F32, name="rec")
                            nc.vector.reciprocal(rec, bcast)
                            # fused normalize + PSUM->SBUF, overwriting head
                            # h's spent q columns
                            nc.vector.tensor_mul(qt[:, h, :], psum_o, rec)

                kvp_cm.__exit__(None, None, None)

                if "C" in phases:
                    # ---- Phase C: o_proj (full Wo); rows are core-owned ----
                    with (
                        tc.tile_pool(name="wop", bufs=56) as wop,
                        tc.tile_pool(name="outp", bufs=3) as outp,
                    ):
                        for dc in range(NDC):
                            wo_tiles = []
                            for ct in range(DKT):
                                wt = wop.tile([128, 512], BF16, name="wo")
                                if ct % 2 == 0:
                                    nc.sync.dma_start(out=wt, in_=wo_d[dc, ct])
                                else:
                                    nc.scalar.dma_start(out=wt, in_=wo_d[dc, ct])
                                wo_tiles.append(wt)
                            for q in range(NQT):
                                psum = ps_proj.tile([128, 512], F32, name="pp")
                                for ct in range(DKT):
                                    nc.tensor.matmul(
                                        psum,
                                        qt[:, ct, q * 128 : (q + 1) * 128],
                                        wo_tiles[ct],
                                        start=(ct == 0),
                                        stop=(ct == DKT - 1),
                                    )
                                ob = outp.tile([128, 512], F32, name="ob")
                                nc.vector.tensor_copy(ob, psum)
                                nc.sync.dma_start(
                                    out=out_d[q, :, dc * 512 : (dc + 1) * 512],
                                    in_=ob,
                                )

    nc.finalize()
    _NC_CACHE[key] = nc
    return nc


def _host_inputs(hidden_states, Wq, Wk, Wv, Wo):
    hidden = np.asarray(hidden_states, dtype=np.float32)
    Wq = np.asarray(Wq, dtype=np.float32) * np.float32(SCALE)
    Wk = np.asarray(Wk, dtype=np.float32)
    Wv = np.asarray(Wv, dtype=np.float32)
    Wo = np.asarray(Wo, dtype=np.float32)

    inv_freq = 1.0 / ROPE_THETA ** (np.arange(0, HD, 2, dtype=np.float32) / HD)
    t = np.arange(S, dtype=np.float32)
    freqs = np.outer(t, inv_freq)  # [S, 64]
    cos_t = np.cos(freqs.T)  # [64, S]
    sin_t = np.sin(freqs.T)
    cosk = np.concatenate([cos_t, cos_t], axis=0).astype(np.float32)  # [128, S]
    sink = np.concatenate([-sin_t, sin_t], axis=0).astype(np.float32)

    # shared weight layouts (identical for every core)
    wq = np.ascontiguousarray(
        Wq.reshape(DKT, 128, H, 128).transpose(2, 1, 0, 3)
    ).astype(BF16_NP)  # [h, p, kt, c]
    wk4 = Wk.reshape(DKT, 128, KVH, 128)
    wv4 = Wv.reshape(DKT, 128, KVH, 128)
    wkv = np.empty((2 * KVH, 2, 128, DKT // 2, 128), BF16_NP)
    for ct in range(KVH):
        for hf in range(2):
            ktsl = slice(hf * (DKT // 2), (hf + 1) * (DKT // 2))
            wkv[ct, hf] = wk4[ktsl, :, ct, :].transpose(1, 0, 2).astype(BF16_NP)
            wkv[KVH + ct, hf] = wv4[ktsl, :, ct, :].transpose(1, 0, 2).astype(BF16_NP)
    wo = np.ascontiguousarray(
        Wo.reshape(DKT, 128, NDC, 512).transpose(2, 0, 1, 3)
    ).astype(BF16_NP)  # [dc, ct, p, d]

    cosk_b = cosk.astype(BF16_NP)
    sink_b = sink.astype(BF16_NP)

    in_maps = []
    for core in range(8):
        b, r = core // 4, core % 4
        tiles = _qtiles(r)
        qpos = np.concatenate(
            [np.arange(t0 * 128, (t0 + 1) * 128) for t0 in tiles]
        )  # [512] ascending global q positions
        xq = np.ascontiguousarray(
            hidden[b][qpos].reshape(QW, DKT, 128).transpose(1, 2, 0)
        ).astype(BF16_NP)  # [kt, p, q]
        xt = np.ascontiguousarray(
            hidden[b].reshape(NSC, SC, DKT, 128).transpose(0, 2, 3, 1)
        ).astype(BF16_NP)
        cosq = np.ascontiguousarray(cosk[:, qpos]).astype(BF16_NP)
        sinq = np.ascontiguousarray(sink[:, qpos]).astype(BF16_NP)
        # mask[kt]: [128, 128] additive mask for the FIRST live block of the
        # suffix (columns QW-w .. QW-w+128). Triangular when that block's
        # q-tile equals kt (the diagonal), all-zero otherwise.
        mask = np.zeros((NKT, 128, 128), np.float32)
        for kti in range(NKT):
            lo = QW - _wof(kti)
            kk = kti * 128 + np.arange(128)[:, None]
            qq = qpos[None, lo : lo + 128]
            mask[kti] = np.where(kk <= qq, 0.0, -30000.0)
        in_maps.append(
            {
                "xq": xq,
                "xt": xt,
                "wq": wq,
                "wkv": wkv,
                "wo": wo,
                "cosq": cosq,
                "sinq": sinq,
                "cosk": cosk_b,
                "sink": sink_b,
                "mask": mask,
            }
        )
    return in_maps


def kernel(hidden_states, Wq, Wk, Wv, Wo, trace=False):
    nc = _build_nc()
    in_maps = _host_inputs(hidden_states, Wq, Wk, Wv, Wo)
    res = run_bass_kernel_spmd(nc, in_maps, list(range(8)), trace=trace)
    out = np.empty((B, S, D), dtype=np.float32)
    for core in range(8):
        b, r = core // 4, core % 4
        o = res.results[core]["out"]  # [NQT, 128, D]
        for j, t0 in enumerate(_qtiles(r)):
            out[b, t0 * 128 : (t0 + 1) * 128, :] = o[j]
    if trace:
        kernel.last_exec_time_ns = res.exec_time_ns
    return out


# revision 6
# speedup vs baseline: 1.3076x; 1.2043x over previous
"""DreamAttention (GQA + RoPE + causal) on 8 trn2 NeuronCores.

v3: sequence-parallel K/V projection + intra-group AllGather.

Sharding: DP=2 over batch x sequence-parallel over q-tiles. Core c ->
(batch b = c // 4, seq rank r = c % 4). Core r owns q-tiles
[r, 7-r, 8+r, 15-r] (128 rows each, ascending); sum of causal work is equal
across ranks. Each core:
  - projects K^T and V (natural layout, no transposes) for ITS OWN 512
    positions only (1/4 of the work v2 did redundantly), ropes K, and
    AllGathers K/V among its 4-core batch group (runs on TOPSP/SDMA silicon,
    overlapped with the Q projection on the PE)
  - projects Q for its 512 positions (28 heads) + RoPE
  - attention in transposed form per (head, k-tile): S^T[k, q-suffix] ->
    exp -> PV accumulates out^T[d, q]; live q columns are a suffix (widths
    512/384/256/128); diagonal via host-built additive mask
  - o_proj: attnT stationary, full Wo moving; output rows are core-owned
All matmul operands BF16 (fp32 PSUM accumulation).
Host reassembles the 8 cores' row-slices into the full [2, 2048, 3584] output.
"""

import math

import numpy as np
import ml_dtypes

import concourse.bass as bass
import concourse.mybir as mybir
import concourse.tile as tile
from concourse import bacc
from concourse.bass_utils import run_bass_kernel_spmd

F32 = mybir.dt.float32
BF16 = mybir.dt.bfloat16
BF16_NP = ml_dtypes.bfloat16

B, S, D = 2, 2048, 3584
H, KVH, HD = 28, 4, 128
ROPE_THETA = 1000000.0
GQ = H // KVH   # 7 q heads per kv head
DKT = D // 128  # 28 k-tiles over D
NKT = S // 128  # 16 k tiles over sequence
NDC = 7         # output D chunks of 512
NQT = 4         # q-tiles owned per core
QW = NQT * 128  # 512 q columns per core
SCALE = 1.0 / math.sqrt(HD)
RG = [[0, 1, 2, 3], [4, 5, 6, 7]]


def _qtiles(r):
    """Ascending q-tile ids owned by seq-rank r; sum of (t+1) == 34 for all r."""
    return [r, 7 - r, 8 + r, 15 - r]


def _wof(kti):
    # Live-suffix width for k-tile kti (rank-independent).
    return 128 * (4 - kti // 4)


# k-tiles packed into 512-wide PSUM bins (sum of live widths == 512 each):
# 4x [512], 4x [384+128], 2x [256+256] -> 10 exp calls per head instead of 16.
BINS = [
    [0], [1], [2], [3],
    [4, 12], [5, 13], [6, 14], [7, 15],
    [8, 9], [10, 11],
]


_NC_CACHE = {}


def _build_nc():
    key = "nc_v3"
    if key in _NC_CACHE:
        return _NC_CACHE[key]

    nc = bacc.Bacc("TRN2", target_bir_lowering=False, debug=False, num_devices=8)

    xq_d = nc.dram_tensor("xq", [DKT, 128, QW], BF16, kind="ExternalInput").ap()
    wq_d = nc.dram_tensor("wq", [H, 128, DKT, 128], BF16, kind="ExternalInput").ap()
    wk_d = nc.dram_tensor("wk", [KVH, 128, DKT, 128], BF16, kind="ExternalInput").ap()
    wv_d = nc.dram_tensor("wv", [DKT, 128, 512], BF16, kind="ExternalInput").ap()
    wo_d = nc.dram_tensor("wo", [NDC, DKT, 128, 512], BF16, kind="ExternalInput").ap()
    cosq_d = nc.dram_tensor("cosq", [128, QW], BF16, kind="ExternalInput").ap()
    sinq_d = nc.dram_tensor("sinq", [128, QW], BF16, kind="ExternalInput").ap()
    mask_d = nc.dram_tensor("mask", [NKT, 128, 128], F32, kind="ExternalInput").ap()
    out_d = nc.dram_tensor("out", [NQT, 128, D], F32, kind="ExternalOutput").ap()

    # collective bounce buffers: [:, 0:4, :] = K^T own (roped), [:, 4:8, :] =
    # V own natural ([128 pos, tile, 512 dv])
    cc_in = nc.dram_tensor("cc_in", [128, 2 * KVH, 512], BF16, kind="Internal").ap()
    cc_out = nc.dram_tensor(
        "cc_out", [4, 128, 2 * KVH, 512], BF16, kind="Internal"
    ).ap()

    with tile.TileContext(nc) as tc:
        with (
            tc.tile_pool(name="persist", bufs=1) as persist,
            tc.tile_pool(name="ps_proj", bufs=2, space="PSUM") as ps_proj,
            tc.tile_pool(name="ps_s", bufs=3, space="PSUM") as ps_s,
            tc.tile_pool(name="ps_o", bufs=2, space="PSUM") as ps_o,
            tc.tile_pool(name="ps_sum", bufs=1, space="PSUM") as ps_sum,
        ):
            # qt doubles as the attention-output buffer: att(h) overwrites
            # qt[:, h, :] once head h's scores are done.
            qt = persist.tile([128, H, QW], BF16, name="qt")
            ones = persist.tile([128, 1], BF16, name="ones")
            xq = persist.tile([128, DKT, QW], BF16, name="xq")
            cosq = persist.tile([128, QW], BF16, name="cosq")
            sinq = persist.tile([128, QW], BF16, name="sinq")
            kt_t = persist.tile([128, KVH, S], BF16, name="kt")
            vn = persist.tile([128, KVH, NKT, 128], BF16, name="vn")

            nc.vector.memset(ones, 1.0)
            nc.scalar.dma_start(out=cosq, in_=cosq_d)
            nc.scalar.dma_start(out=sinq, in_=sinq_d)
            nc.scalar.dma_start(out=xq, in_=xq_d.rearrange("k p q -> p k q"))

            def rope(dst, cos_ap, sin_ap, width, tmp):
                t = tmp[:, :width]
                nc.scalar.dma_start(out=t[0:64, :], in_=dst[64:128, :])
                nc.scalar.dma_start(out=t[64:128, :], in_=dst[0:64, :])
                nc.vector.tensor_mul(t, t, sin_ap)
                nc.vector.tensor_mul(dst, dst, cos_ap)
                nc.vector.tensor_add(dst, dst, t)

            # ---- Phase A0: own-position K/V projection + K-RoPE + AllGather
            with (
                tc.tile_pool(name="kvw", bufs=1) as kvw,
                tc.tile_pool(name="kvown", bufs=1) as kvown,
                tc.tile_pool(name="krtmp", bufs=2) as krtmp,
            ):
                wv_t = kvw.tile([128, DKT, 512], BF16, name="wv_t")
                nc.sync.dma_start(out=wv_t, in_=wv_d.rearrange("k p d -> p k d"))
                wk_t = kvw.tile([128, KVH, DKT, 128], BF16, name="wk_t")
                nc.sync.dma_start(out=wk_t, in_=wk_d.rearrange("h p k c -> p h k c"))

                kt_own = kvown.tile([128, KVH, QW], BF16, name="kt_own")
                v_own = kvown.tile([128, NQT, 512], BF16, name="v_own")

                for kv in range(KVH):
                    psum = ps_proj.tile([128, QW], F32, name="pp")
                    for kti in range(DKT):
                        nc.tensor.matmul(
                            psum,
                            wk_t[:, kv, kti, :],
                            xq[:, kti, :],
                            start=(kti == 0),
                            stop=(kti == DKT - 1),
                        )
                    nc.vector.tensor_copy(kt_own[:, kv, :], psum)
                    tmp = krtmp.tile([128, QW], BF16, name="krtmp")
                    rope(kt_own[:, kv, :], cosq, sinq, QW, tmp)
                for ti in range(NQT):
                    psum = ps_proj.tile([128, 512], F32, name="pp")
                    for kti in range(DKT):
                        nc.tensor.matmul(
                            psum,
                            xq[:, kti, ti * 128 : (ti + 1) * 128],
                            wv_t[:, kti, :],
                            start=(kti == 0),
                            stop=(kti == DKT - 1),
                        )
                    nc.vector.tensor_copy(v_own[:, ti, :], psum)

                nc.gpsimd.dma_start(out=cc_in[:, 0:KVH, :], in_=kt_own)
                nc.gpsimd.dma_start(out=cc_in[:, KVH:, :], in_=v_own)
                nc.gpsimd.collective_compute(
                    "AllGather",
                    mybir.AluOpType.bypass,
                    replica_groups=RG,
                    ins=[cc_in],
                    outs=[cc_out],
                )
                # unpack (waits on AG on the sync queue; overlaps Q proj on PE)
                for r in range(4):
                    for ti, t0 in enumerate(_qtiles(r)):
                        nc.sync.dma_start(
                            out=kt_t[:, :, t0 * 128 : (t0 + 1) * 128],
                            in_=cc_out[r, :, 0:KVH, ti * 128 : (ti + 1) * 128],
                        )
                        nc.sync.dma_start(
                            out=vn[:, :, t0, :],
                            in_=cc_out[r, :, KVH + ti, :].rearrange(
                                "p (kv c) -> p kv c", kv=KVH
                            ),
                        )

            # ---- Phase A1: Q projection + fused Q-RoPE ----
            with (
                tc.tile_pool(name="wqp", bufs=3) as wqp,
                tc.tile_pool(name="qrtmp", bufs=3) as qrtmp,
            ):
                for ct in range(H):
                    wblk = wqp.tile([128, DKT, 128], BF16, name="wq")
                    nc.scalar.dma_start(out=wblk, in_=wq_d[ct])
                    psum = ps_proj.tile([128, QW], F32, name="pp")
                    for kti in range(DKT):
                        nc.tensor.matmul(
                            psum,
                            wblk[:, kti, :],
                            xq[:, kti, :],
                            start=(kti == 0),
                            stop=(kti == DKT - 1),
                        )
                    nc.vector.tensor_copy(qt[:, ct, :], psum)
                    tmp = qrtmp.tile([128, QW], BF16, name="qrtmp")
                    rope(qt[:, ct, :], cosq, sinq, QW, tmp)

            # ---- Phase B: attention, 28 heads on the core's 512 q ----
            with (
                tc.tile_pool(name="ptp", bufs=3) as ptp,
                tc.tile_pool(name="smallp", bufs=2) as smallp,
                tc.tile_pool(name="maskp", bufs=1) as maskp,
            ):
                mask_t = maskp.tile([128, NKT, 128], F32, name="mask_t")
                nc.scalar.dma_start(
                    out=mask_t, in_=mask_d.rearrange("k p q -> p k q")
                )
                for h in range(H):
                    kv = h // GQ
                    psum_o = ps_o.tile([128, QW], F32, name="po")
                    psum_r = ps_sum.tile([1, QW], F32, name="psr")
                    sub_idx = 0
                    for bin_tiles in BINS:
                        psum_s = ps_s.tile([128, 512], F32, name="pss")
                        off = 0
                        for kti in bin_tiles:
                            w = _wof(kti)
                            lo = QW - w
                            nc.tensor.matmul(
                                psum_s[:, off : off + w],
                                kt_t[:, kv, kti * 128 : (kti + 1) * 128],
                                qt[:, h, lo:],
                                start=True,
                                stop=True,
                            )
                            # first live block of each sub-tile may be
                            # diagonal; host mask is zero elsewhere
                            nc.vector.tensor_add(
                                psum_s[:, off : off + 128],
                                psum_s[:, off : off + 128],
                                mask_t[:, kti, :],
                            )
                            off += w
                        pt = ptp.tile([128, 512], BF16, name="pt")
                        nc.scalar.activation(
                            pt,
                            psum_s,
                            mybir.ActivationFunctionType.Exp,
                        )
                        off = 0
                        for kti in bin_tiles:
                            w = _wof(kti)
                            lo = QW - w
                            nc.tensor.matmul(
                                psum_o[:, lo:],
                                vn[:, kv, kti, :],
                                pt[:, off : off + w],
                                start=(sub_idx == 0),
                                stop=(sub_idx == NKT - 1),
                            )
                            nc.tensor.matmul(
                                psum_r[:, lo:],
                                ones,
                                pt[:, off : off + w],
                                start=(sub_idx == 0),
                                stop=(sub_idx == NKT - 1),
                            )
                            off += w
                            sub_idx += 1
                    sum_sb = smallp.tile([1, QW], F32, name="sum_sb")
                    nc.scalar.copy(sum_sb, psum_r)
                    bcast = smallp.tile([128, QW], F32, name="bcast")
                    nc.gpsimd.partition_broadcast(bcast, sum_sb)
                    rec = smallp.tile([128, QW], F32, name="rec")
                    nc.vector.reciprocal(rec, bcast)
                    # fused normalize + PSUM->SBUF, overwriting head h's
                    # spent q columns
                    nc.vector.tensor_mul(qt[:, h, :], psum_o, rec)

            # ---- Phase C: o_proj (full Wo); rows are core-owned ----
            with (
                tc.tile_pool(name="wop", bufs=56) as wop,
                tc.tile_pool(name="outp", bufs=3) as outp,
            ):
                for dc in range(NDC):
                    wo_tiles = []
                    for ct in range(DKT):
                        wt = wop.tile([128, 512], BF16, name="wo")
                        if ct % 2 == 0:
                            nc.sync.dma_start(out=wt, in_=wo_d[dc, ct])
                        else:
                            nc.scalar.dma_start(out=wt, in_=wo_d[dc, ct])
                        wo_tiles.append(wt)
                    for q in range(NQT):
                        psum = ps_proj.tile([128, 512], F32, name="pp")
                        for ct in range(DKT):
                            nc.tensor.matmul(
                                psum,
                                qt[:, ct, q * 128 : (q + 1) * 128],
                                wo_tiles[ct],
                                start=(ct == 0),
                                stop=(ct == DKT - 1),
                            )
                        ob = outp.tile([128, 512], F32, name="ob")
                        nc.vector.tensor_copy(ob, psum)
                        nc.sync.dma_start(
                            out=out_d[q, :, dc * 512 : (dc + 1) * 512],
                            in_=ob,
                        )

    nc.finalize()
    _NC_CACHE[key] = nc
    return nc


def _host_inputs(hidden_states, Wq, Wk, Wv, Wo):
    hidden = np.asarray(hidden_states, dtype=np.float32)
    Wq = np.asarray(Wq, dtype=np.float32) * np.float32(SCALE)
    Wk = np.asarray(Wk, dtype=np.float32)
    Wv = np.asarray(Wv, dtype=np.float32)
    Wo = np.asarray(Wo, dtype=np.float32)

    inv_freq = 1.0 / ROPE_THETA ** (np.arange(0, HD, 2, dtype=np.float32) / HD)
    t = np.arange(S, dtype=np.float32)
    freqs = np.outer(t, inv_freq)  # [S, 64]
    cos_t = np.cos(freqs.T)  # [64, S]
    sin_t = np.sin(freqs.T)
    cosk = np.concatenate([cos_t, cos_t], axis=0).astype(np.float32)  # [128, S]
    sink = np.concatenate([-sin_t, sin_t], axis=0).astype(np.float32)

    # shared weight layouts (identical for every core)
    wq = np.ascontiguousarray(
        Wq.reshape(DKT, 128, H, 128).transpose(2, 1, 0, 3)
    ).astype(BF16_NP)  # [h, p, kt, c]
    wk = np.ascontiguousarray(
        Wk.reshape(DKT, 128, KVH, 128).transpose(2, 1, 0, 3)
    ).astype(BF16_NP)  # [kv, p, kt, c]
    wv = np.ascontiguousarray(Wv.reshape(DKT, 128, KVH * 128)).astype(BF16_NP)
    wo = np.ascontiguousarray(
        Wo.reshape(DKT, 128, NDC, 512).transpose(2, 0, 1, 3)
    ).astype(BF16_NP)  # [dc, ct, p, d]

    in_maps = []
    for core in range(8):
        b, r = core // 4, core % 4
        tiles = _qtiles(r)
        qpos = np.concatenate(
            [np.arange(t0 * 128, (t0 + 1) * 128) for t0 in tiles]
        )  # [512] ascending global q positions
        xq = np.ascontiguousarray(
            hidden[b][qpos].reshape(QW, DKT, 128).transpose(1, 2, 0)
        ).astype(BF16_NP)  # [kt, p, q]
        cosq = np.ascontiguousarray(cosk[:, qpos]).astype(BF16_NP)
        sinq = np.ascontiguousarray(sink[:, qpos]).astype(BF16_NP)
        # mask[kt]: [128, 128] additive mask for the FIRST live block of the
        # suffix (triangular when that block's q-tile equals kt, else zero).
        mask = np.zeros((NKT, 128, 128), np.float32)
        for kti in range(NKT):
            lo = QW - _wof(kti)
            kk = kti * 128 + np.arange(128)[:, None]
            qq = qpos[None, lo : lo + 128]
            mask[kti] = np.where(kk <= qq, 0.0, -30000.0)
        in_maps.append(
            {
                "xq": xq,
                "wq": wq,
                "wk": wk,
                "wv": wv,
                "wo": wo,
                "cosq": cosq,
                "sinq": sinq,
                "mask": mask,
            }
        )
    return in_maps


def kernel(hidden_states, Wq, Wk, Wv, Wo, trace=False):
    nc = _build_nc()
    in_maps = _host_inputs(hidden_states, Wq, Wk, Wv, Wo)
    res = run_bass_kernel_spmd(nc, in_maps, list(range(8)), trace=trace)
    out = np.empty((B, S, D), dtype=np.float32)
    for core in range(8):
        b, r = core // 4, core % 4
        o = res.results[core]["out"]  # [NQT, 128, D]
        for j, t0 in enumerate(_qtiles(r)):
            out[b, t0 * 128 : (t0 + 1) * 128, :] = o[j]
    if trace:
        kernel.last_exec_time_ns = res.exec_time_ns
    return out
